# revision 15
# baseline (speedup 1.0000x reference)
"""Dense CRF pairwise loss on 8 Trainium2 NeuronCores.

loss = (2/N) * [ sum_{i<j} (a_i b_j + a_j b_i) K_ij + sum_i a_i b_i ],
a = probs[:,0], b = 1-a, K_ij = exp(-c1*d_xy - c2*d_rgb), K_ii = 1.

Pixels are permuted into 8x16 patches (i-tiles of 128) grouped into 16x32
chunks (j-columns of 512).  The device computes the strictly-off-diagonal
upper-triangle blocks (i-tile t vs chunk m, t < 4m) whose patch boxes are
within RCUT=33 px (the Gaussian tail beyond is negligible): 55 slots/core.
The 18 block-diagonal 512x512 triangles and the K_ii=1 diagonal are done
on host in numpy (~3% of pairs) -- no masks needed on device.

The exponent x = c1*d_xy + c2*d_rgb is ONE fp8e4m3 matmul per block:
features are pre-scaled by sqrt(c1) and hi/mid/lo-split so every value is
e4m3-exact; rank 36, zero-padded to a 96-partition contraction because the
PE runs 2 cols/cycle only when the contraction dim is >= ~96 (measured:
K=96 -> 242 ns per 512-col matmul, K<=88 -> 443 ns).  ScalarE applies
exp(-x + ln 128) writing K~ = 128*K in fp8 (the 2^7 scale preserves
small-K mass against the e4m3 subnormal floor).

Per column, per-slot stats [ah al bh bl] (fp8, 16B-padded) reduce K~ over
i with fp8 DoubleRow r-matmuls, two slots per matmul ([128,2,4]^T @
[128,2,512] -> [4,512], 248 ns = 2 fp8 cols/cycle); odd tails use a plain
fp8 matmul.  DVE copies each column's [4,512] PSUM accumulator to a stage
buffer; the stage is DMAd out and dotted with [b,b,a,a] on host.

SPMD: all cores run the identical 55-slot schedule; slot -> i-tile is
per-core data.  Padding slots have zero features and zero stats
(exp -> 128, stats 0 -> no contribution).
"""

import numpy as np
import ml_dtypes

import concourse.bass as bass
import concourse.tile as tile
from concourse import bacc, mybir
from concourse.bass_utils import run_bass_kernel_spmd

E4 = ml_dtypes.float8_e4m3

H = W = 96
N = H * W                       # 9216
N_CORES = 8
JC = 512                        # column width (one 16x32 chunk)
N_CHUNKS = 18
IT = 128                        # i-tile (8x16 patch)
KPART = 96                      # matmul contraction partitions (36 real)
RANK = 36
RCUT = 33.0                     # patch-box cull radius (px)
KSCALE = 128.0                  # K stored as 128*K in fp8
LN_KSCALE = float(np.log(KSCALE))

SIGMA_XY = 15.0
SIGMA_RGB = 0.125
C1 = 1.0 / (2.0 * SIGMA_XY * SIGMA_XY)
C2 = 1.0 / (2.0 * SIGMA_RGB * SIGMA_RGB)
LAM = np.sqrt(C2 / C1)          # 120

_CACHE = {}


# ---------------- geometry: patches, chunks, cull, schedule ----------------

def _geometry():
    boxes = []          # per patch (y0,y1,x0,x1) inclusive
    perm = []           # new pixel index -> original row-major index
    for cy in range(6):
        for cx in range(3):
            for py in range(2):
                for px in range(2):
                    y0, x0 = cy * 16 + py * 8, cx * 32 + px * 16
                    boxes.append((y0, y0 + 7, x0, x0 + 15))
                    for yy in range(y0, y0 + 8):
                        for xx in range(x0, x0 + 16):
                            perm.append(yy * 96 + xx)
    perm = np.array(perm)
    cbox = [(min(boxes[4 * m + q][0] for q in range(4)),
             max(boxes[4 * m + q][1] for q in range(4)),
             min(boxes[4 * m + q][2] for q in range(4)),
             max(boxes[4 * m + q][3] for q in range(4))) for m in range(N_CHUNKS)]

    def bdist(b1, b2):
        dy = max(0, b1[0] - b2[1], b2[0] - b1[1])
        dx = max(0, b1[2] - b2[3], b2[2] - b1[3])
        return float(np.hypot(dy, dx))

    # strictly-off-diagonal kept blocks only (t < 4m); diagonal on host
    kept = {m: [t for t in range(4 * m)
                if bdist(boxes[t], cbox[m]) <= RCUT] for m in range(N_CHUNKS)}
    cols = [m for m in range(N_CHUNKS) if kept[m]]
    s_m = {m: -(-len(kept[m]) // N_CORES) for m in cols}
    # big columns first (DMA streaming + short tail), small columns last
    col_order = sorted(cols, key=lambda m: -s_m[m])
    s_o = [s_m[m] for m in col_order]

    assign = {}
    for m in cols:
        A = -np.ones((N_CORES, s_m[m]), int)
        for k, t in enumerate(kept[m]):
            A[k % N_CORES, k // N_CORES] = t
        assign[m] = A
    return perm, col_order, s_o, assign


PERM, COL_ORDER, S_O, ASSIGN = _geometry()
N_COLS = len(COL_ORDER)         # 17
NSLOTS = sum(S_O)               # 55


# ---------------- device program ----------------

def _build_program():
    nc = bacc.Bacc("TRN2", target_bir_lowering=False, debug=False)
    f32 = mybir.dt.float32
    fp8 = mybir.dt.float8e4
    DR = mybir.MatmulPerfMode.DoubleRow

    uf_d = nc.dram_tensor("uf", [KPART, NSLOTS * IT], fp8, kind="ExternalInput")
    vf_d = nc.dram_tensor("vf", [KPART, N], fp8, kind="ExternalInput")
    st_d = nc.dram_tensor("st", [128, NSLOTS, 16], fp8, kind="ExternalInput")
    stage_d = nc.dram_tensor("stage", [4, N_COLS * JC], f32, kind="ExternalOutput")

    base_o = np.concatenate([[0], np.cumsum(S_O)]).astype(int)

    with tile.TileContext(nc) as tc:
        with (
            tc.tile_pool(name="const", bufs=1) as cpool,
            tc.tile_pool(name="kgrp", bufs=2) as kpool,
            tc.tile_pool(name="pse", bufs=6, space="PSUM") as pe_pool,
            tc.tile_pool(name="psr", bufs=2, space="PSUM") as pr_pool,
        ):
            uf_t = cpool.tile([KPART, NSLOTS * IT], fp8)
            vf_t = cpool.tile([KPART, N], fp8)
            st_t = cpool.tile([128, NSLOTS, 16], fp8)
            stage_t = cpool.tile([4, N_COLS * JC], f32)
            warm_t = cpool.tile([KPART, JC], fp8)
            bias_t = cpool.tile([128, 1], f32)

            # --- input DMAs (sync/gpsimd/scalar queues), first-needed first
            nc.gpsimd.dma_start(uf_t[:, 0:3 * IT], uf_d.ap()[:, 0:3 * IT])
            nc.gpsimd.dma_start(uf_t[:, 3 * IT:10 * IT],
                                uf_d.ap()[:, 3 * IT:10 * IT])
            nc.gpsimd.dma_start(uf_t[:, 10 * IT:28 * IT],
                                uf_d.ap()[:, 10 * IT:28 * IT])
            nc.gpsimd.dma_start(uf_t[:, 28 * IT:], uf_d.ap()[:, 28 * IT:])
            nc.sync.dma_start(vf_t[:, 0:JC], vf_d.ap()[:, 0:JC])
            nc.sync.dma_start(vf_t[:, JC:4 * JC], vf_d.ap()[:, JC:4 * JC])
            nc.sync.dma_start(st_t[:, 0:8, :], st_d.ap()[:, 0:8, :])
            nc.sync.dma_start(st_t[:, 8:, :], st_d.ap()[:, 8:, :])
            nc.scalar.dma_start(vf_t[:, 4 * JC:], vf_d.ap()[:, 4 * JC:])
            nc.vector.memset(warm_t[:], 0.0)
            nc.vector.memset(bias_t[:], LN_KSCALE)

            # --- PE warm-up while DMAs land ---
            warm_ps = pe_pool.tile([128, 1, JC], f32, tag="pse", name="warm_ps")
            for _ in range(7):
                nc.tensor.matmul(warm_ps[:, 0, :], warm_t[:, 0:IT], warm_t[:],
                                 start=True, stop=True)

            # --- main pipeline: columns in order, r-matmuls one column behind
            pending = []
            done = [0]

            def flush(o, s, kbuf, psr_t):
                npair = s // 2
                for p in range(npair):
                    g = base_o[o] + 2 * p
                    nc.tensor.matmul(
                        psr_t[:], st_t[:, g:g + 2, 0:4], kbuf[:, 2 * p:2 * p + 2, :],
                        start=(p == 0), stop=(p == npair - 1 and s % 2 == 0),
                        perf_mode=DR)
                if s % 2:
                    g = base_o[o] + s - 1
                    nc.tensor.matmul(
                        psr_t[:], st_t[:, g, 0:4], kbuf[:, s - 1, :],
                        start=(s == 1), stop=True)
                nc.vector.tensor_copy(stage_t[:, o * JC:(o + 1) * JC], psr_t[:])
                nc.sync.dma_start(stage_d.ap()[:, o * JC:(o + 1) * JC],
                                  stage_t[:, o * JC:(o + 1) * JC])
                done[0] += 1

            for o in range(N_COLS):
                s = S_O[o]
                kbuf = kpool.tile([128, 5, JC], fp8, tag="kg", name=f"kb{o}")
                psr_t = pr_pool.tile([4, JC], f32, tag="psr", name=f"pr{o}")
                for g0 in range(0, s, 1):
                    ln = min(1, s - g0)
                    ps = pe_pool.tile([128, 1, JC], f32, tag="pse")
                    for u in range(ln):
                        gslot = base_o[o] + g0 + u
                        nc.tensor.matmul(
                            ps[:, u, :],
                            uf_t[:, gslot * IT:(gslot + 1) * IT],
                            vf_t[:, o * JC:(o + 1) * JC],
                            start=True, stop=True)
                    nc.scalar.activation(
                        kbuf[:, g0:g0 + ln, :], ps[:, 0:ln, :],
                        mybir.ActivationFunctionType.Exp,
                        scale=-1.0, bias=bias_t[:, 0:1])
                pending.append((o, s, kbuf, psr_t))
                if len(pending) > 1:
                    flush(*pending.pop(0))
            while pending:
                flush(*pending.pop(0))

    nc.compile()
    return nc


# ---------------- host-side features ----------------

def _f8(v):
    return np.asarray(np.asarray(v).astype(E4), np.float64)


def _features(probs, image):
    ys, xs = np.meshgrid(np.arange(H, dtype=np.float64),
                         np.arange(W, dtype=np.float64), indexing="ij")
    y = ys.ravel()[PERM]
    x = xs.ravel()[PERM]
    col = image[0].astype(np.float64).reshape(3, N)[:, PERM]
    a = probs[0, 0].astype(np.float64).reshape(N)[PERM]
    b = 1.0 - a

    rC1 = np.sqrt(C1)
    yt, xt, gt = rC1 * y, rC1 * x, (rC1 * LAM) * col
    base = yt * yt + xt * xt + (gt * gt).sum(axis=0)
    B1 = _f8(base); B2 = _f8(base - B1); B3 = _f8(base - B1 - B2)
    one = np.ones(N)
    U, V = [], []
    for t in (B1, B2, B3):
        U.append(t); V.append(one)
    for t in (B1, B2, B3):
        U.append(one); V.append(t)

    def cross(w):
        h = _f8(w); r = w - h; m = _f8(r); l = _f8(r - m)
        for ui, vj in [(h, h), (h, m), (m, h), (h, l), (l, h), (m, m)]:
            U.append(_f8(-2.0 * ui)); V.append(vj)

    cross(yt); cross(xt)
    for ch in range(3):
        cross(gt[ch])
    U = np.stack(U).astype(E4)      # [36, N]
    V = np.stack(V).astype(E4)

    ah = _f8(a); al = _f8(a - ah); bh = _f8(b); bl = _f8(b - bh)
    stat = np.stack([ah, al, bh, bl], axis=1).astype(E4)   # [N, 4]
    return U, V, stat, a, b, y, x, col


def _host_diag(y, x, col, a, b):
    """K_ii diagonal plus the 18 in-chunk 512x512 upper triangles (fp64)."""
    tot = float((a * b).sum())
    iu = np.triu_indices(JC, k=1)
    for m in range(N_CHUNKS):
        sl = slice(m * JC, (m + 1) * JC)
        yy, xx, aa, bb = y[sl], x[sl], a[sl], b[sl]
        cc = col[:, sl]
        dxy = (yy[:, None] - yy[None, :]) ** 2 + (xx[:, None] - xx[None, :]) ** 2
        drgb = ((cc[:, :, None] - cc[:, None, :]) ** 2).sum(axis=0)
        K = np.exp(-C1 * dxy - C2 * drgb)
        w = aa[:, None] * bb[None, :] + bb[:, None] * aa[None, :]
        tot += float((w[iu] * K[iu]).sum())
    return tot


def kernel(probs: np.ndarray, image: np.ndarray) -> np.ndarray:
    probs = np.asarray(probs)
    image = np.asarray(image)
    assert probs.shape == (1, 2, H, W) and image.shape == (1, 3, H, W)

    if "nc" not in _CACHE:
        _CACHE["nc"] = _build_program()
    nc = _CACHE["nc"]

    U, V, stat, a, b, y, x, col = _features(probs, image)

    vf = np.zeros((KPART, N), dtype=E4)
    for o, m in enumerate(COL_ORDER):
        vf[:RANK, o * JC:(o + 1) * JC] = V[:, m * JC:(m + 1) * JC]

    in_maps = []
    for c in range(N_CORES):
        uf = np.zeros((KPART, NSLOTS * IT), dtype=E4)
        st = np.zeros((128, NSLOTS, 16), dtype=E4)
        g = 0
        for o, m in enumerate(COL_ORDER):
            for s in range(S_O[o]):
                t = ASSIGN[m][c, s]
                if t >= 0:
                    iw = slice(t * IT, (t + 1) * IT)
                    uf[:RANK, g * IT:(g + 1) * IT] = U[:, iw]
                    st[:, g, 0:4] = stat[iw, :]
                g += 1
        in_maps.append({"uf": uf, "vf": vf, "st": st})
    _CACHE["in_maps"] = in_maps

    res = run_bass_kernel_spmd(nc, in_maps, list(range(N_CORES)))

    tri = np.float64(0.0)
    for c in range(N_CORES):
        stage = res.results[c]["stage"].astype(np.float64)   # [4, 17*512]
        for o, m in enumerate(COL_ORDER):
            jw = slice(m * JC, (m + 1) * JC)
            r = stage[:, o * JC:(o + 1) * JC]
            tri += ((r[0] + r[1]) * b[jw]).sum() + ((r[2] + r[3]) * a[jw]).sum()

    tri /= KSCALE
    tri += _host_diag(y, x, col, a, b)
    loss = 2.0 * tri / N
    return np.float32(loss)


# revision 16
# speedup vs baseline: 1.0638x; 1.0638x over previous
"""Dense CRF pairwise loss on 8 Trainium2 NeuronCores.

loss = (2/N) * [ sum_{i<j} (a_i b_j + a_j b_i) K_ij + sum_i a_i b_i ],
a = probs[:,0], b = 1-a, K_ij = exp(-c1*d_xy - c2*d_rgb), K_ii = 1.

Pixels are permuted into 8x16 patches (i-tiles of 128) grouped into 16x32
chunks (j-columns of 512).  The device computes the strictly-off-diagonal
upper-triangle blocks (i-tile t vs chunk m, t < 4m) whose patch boxes are
within RCUT=33 px (the Gaussian tail beyond is negligible): 55 slots/core.
The 18 block-diagonal 512x512 triangles and the K_ii=1 diagonal are done
on host in numpy (~3% of pairs) -- no masks needed on device.

The exponent x = c1*d_xy + c2*d_rgb is ONE fp8e4m3 matmul per block:
features are pre-scaled by sqrt(c1) and hi/mid/lo-split so every value is
e4m3-exact; rank 36, zero-padded to a 96-partition contraction because the
PE runs 2 cols/cycle only when the contraction dim is >= ~96 (measured:
K=96 -> 242 ns per 512-col matmul, K<=88 -> 443 ns).  ScalarE applies
exp(-x + ln 128) writing K~ = 128*K in fp8 (the 2^7 scale preserves
small-K mass against the e4m3 subnormal floor).

Per column, per-slot stats [ah al bh bl] (fp8, 16B-padded) reduce K~ over
i with fp8 DoubleRow r-matmuls, two slots per matmul ([128,2,4]^T @
[128,2,512] -> [4,512], 248 ns = 2 fp8 cols/cycle); odd tails use a plain
fp8 matmul.  DVE copies each column's [4,512] PSUM accumulator to a stage
buffer; the stage is DMAd out and dotted with [b,b,a,a] on host.

SPMD: all cores run the identical 55-slot schedule; slot -> i-tile is
per-core data.  Padding slots have zero features and zero stats
(exp -> 128, stats 0 -> no contribution).
"""

import numpy as np
import ml_dtypes

import concourse.bass as bass
import concourse.tile as tile
from concourse import bacc, mybir
from concourse.bass_utils import run_bass_kernel_spmd

E4 = ml_dtypes.float8_e4m3

H = W = 96
N = H * W                       # 9216
N_CORES = 8
JC = 512                        # column width (one 16x32 chunk)
N_CHUNKS = 18
IT = 128                        # i-tile (8x16 patch)
KPART = 96                      # matmul contraction partitions (36 real)
RANK = 36
RCUT = 33.0                     # patch-box cull radius (px)
KSCALE = 128.0                  # K stored as 128*K in fp8
LN_KSCALE = float(np.log(KSCALE))

SIGMA_XY = 15.0
SIGMA_RGB = 0.125
C1 = 1.0 / (2.0 * SIGMA_XY * SIGMA_XY)
C2 = 1.0 / (2.0 * SIGMA_RGB * SIGMA_RGB)
LAM = np.sqrt(C2 / C1)          # 120

_CACHE = {}


# ---------------- geometry: patches, chunks, cull, schedule ----------------

def _geometry():
    boxes = []          # per patch (y0,y1,x0,x1) inclusive
    perm = []           # new pixel index -> original row-major index
    for cy in range(6):
        for cx in range(3):
            for py in range(2):
                for px in range(2):
                    y0, x0 = cy * 16 + py * 8, cx * 32 + px * 16
                    boxes.append((y0, y0 + 7, x0, x0 + 15))
                    for yy in range(y0, y0 + 8):
                        for xx in range(x0, x0 + 16):
                            perm.append(yy * 96 + xx)
    perm = np.array(perm)
    cbox = [(min(boxes[4 * m + q][0] for q in range(4)),
             max(boxes[4 * m + q][1] for q in range(4)),
             min(boxes[4 * m + q][2] for q in range(4)),
             max(boxes[4 * m + q][3] for q in range(4))) for m in range(N_CHUNKS)]

    def bdist(b1, b2):
        dy = max(0, b1[0] - b2[1], b2[0] - b1[1])
        dx = max(0, b1[2] - b2[3], b2[2] - b1[3])
        return float(np.hypot(dy, dx))

    # strictly-off-diagonal kept blocks only (t < 4m); diagonal on host
    kept = {m: [t for t in range(4 * m)
                if bdist(boxes[t], cbox[m]) <= RCUT] for m in range(N_CHUNKS)}
    cols = [m for m in range(N_CHUNKS) if kept[m]]
    s_m = {m: -(-len(kept[m]) // N_CORES) for m in cols}
    # big columns first (DMA streaming + short tail), small columns last
    col_order = sorted(cols, key=lambda m: -s_m[m])
    s_o = [s_m[m] for m in col_order]

    assign = {}
    for m in cols:
        A = -np.ones((N_CORES, s_m[m]), int)
        for k, t in enumerate(kept[m]):
            A[k % N_CORES, k // N_CORES] = t
        assign[m] = A
    return perm, col_order, s_o, assign


PERM, COL_ORDER, S_O, ASSIGN = _geometry()
N_COLS = len(COL_ORDER)         # 17
NSLOTS = sum(S_O)               # 55


# ---------------- device program ----------------

def _build_program():
    nc = bacc.Bacc("TRN2", target_bir_lowering=False, debug=False)
    f32 = mybir.dt.float32
    fp8 = mybir.dt.float8e4
    DR = mybir.MatmulPerfMode.DoubleRow

    uf_d = nc.dram_tensor("uf", [KPART, NSLOTS * IT], fp8, kind="ExternalInput")
    vf_d = nc.dram_tensor("vf", [KPART, N], fp8, kind="ExternalInput")
    st_d = nc.dram_tensor("st", [128, NSLOTS, 16], fp8, kind="ExternalInput")
    stage_d = nc.dram_tensor("stage", [4, N_COLS * JC], f32, kind="ExternalOutput")

    base_o = np.concatenate([[0], np.cumsum(S_O)]).astype(int)

    with tile.TileContext(nc) as tc:
        with (
            tc.tile_pool(name="const", bufs=1) as cpool,
            tc.tile_pool(name="kgrp", bufs=2) as kpool,
            tc.tile_pool(name="pse", bufs=3, space="PSUM") as pe_pool,
            tc.tile_pool(name="psr", bufs=2, space="PSUM") as pr_pool,
        ):
            uf_t = cpool.tile([KPART, NSLOTS * IT], fp8)
            vf_t = cpool.tile([KPART, N], fp8)
            st_t = cpool.tile([128, NSLOTS, 16], fp8)
            stage_t = cpool.tile([4, N_COLS * JC], f32)
            warm_t = cpool.tile([KPART, JC], fp8)
            bias_t = cpool.tile([128, 1], f32)

            # --- input DMAs (sync/gpsimd/scalar queues), first-needed first
            nc.gpsimd.dma_start(uf_t[:, 0:3 * IT], uf_d.ap()[:, 0:3 * IT])
            nc.gpsimd.dma_start(uf_t[:, 3 * IT:10 * IT],
                                uf_d.ap()[:, 3 * IT:10 * IT])
            nc.gpsimd.dma_start(uf_t[:, 10 * IT:28 * IT],
                                uf_d.ap()[:, 10 * IT:28 * IT])
            nc.gpsimd.dma_start(uf_t[:, 28 * IT:], uf_d.ap()[:, 28 * IT:])
            nc.sync.dma_start(vf_t[:, 0:JC], vf_d.ap()[:, 0:JC])
            nc.sync.dma_start(vf_t[:, JC:4 * JC], vf_d.ap()[:, JC:4 * JC])
            nc.sync.dma_start(st_t[:, 0:8, :], st_d.ap()[:, 0:8, :])
            nc.sync.dma_start(st_t[:, 8:, :], st_d.ap()[:, 8:, :])
            nc.scalar.dma_start(vf_t[:, 4 * JC:], vf_d.ap()[:, 4 * JC:])
            nc.vector.memset(warm_t[:], 0.0)
            nc.vector.memset(bias_t[:], LN_KSCALE)

            # --- PE warm-up while DMAs land ---
            warm_ps = pe_pool.tile([128, 2, JC], f32, tag="pse", name="warm_ps")
            for _ in range(7):
                nc.tensor.matmul(warm_ps[:, 0, :], warm_t[:, 0:IT], warm_t[:],
                                 start=True, stop=True)

            # --- main pipeline: columns in order, r-matmuls one column behind
            pending = []
            done = [0]

            def flush(o, s, kbuf, psr_t):
                npair = s // 2
                for p in range(npair):
                    g = base_o[o] + 2 * p
                    nc.tensor.matmul(
                        psr_t[:], st_t[:, g:g + 2, 0:4], kbuf[:, 2 * p:2 * p + 2, :],
                        start=(p == 0), stop=(p == npair - 1 and s % 2 == 0),
                        perf_mode=DR)
                if s % 2:
                    g = base_o[o] + s - 1
                    nc.tensor.matmul(
                        psr_t[:], st_t[:, g, 0:4], kbuf[:, s - 1, :],
                        start=(s == 1), stop=True)
                nc.vector.tensor_copy(stage_t[:, o * JC:(o + 1) * JC], psr_t[:])
                nc.sync.dma_start(stage_d.ap()[:, o * JC:(o + 1) * JC],
                                  stage_t[:, o * JC:(o + 1) * JC])
                done[0] += 1

            for o in range(N_COLS):
                s = S_O[o]
                kbuf = kpool.tile([128, 5, JC], fp8, tag="kg", name=f"kb{o}")
                psr_t = pr_pool.tile([4, JC], f32, tag="psr", name=f"pr{o}")
                for g0 in range(0, s, 2):
                    ln = min(2, s - g0)
                    ps = pe_pool.tile([128, 2, JC], f32, tag="pse")
                    for u in range(ln):
                        gslot = base_o[o] + g0 + u
                        nc.tensor.matmul(
                            ps[:, u, :],
                            uf_t[:, gslot * IT:(gslot + 1) * IT],
                            vf_t[:, o * JC:(o + 1) * JC],
                            start=True, stop=True)
                    nc.scalar.activation(
                        kbuf[:, g0:g0 + ln, :], ps[:, 0:ln, :],
                        mybir.ActivationFunctionType.Exp,
                        scale=-1.0, bias=bias_t[:, 0:1])
                pending.append((o, s, kbuf, psr_t))
                if len(pending) > 1:
                    flush(*pending.pop(0))
            while pending:
                flush(*pending.pop(0))

    nc.compile()
    return nc


# ---------------- host-side features ----------------

def _f8(v):
    return np.asarray(np.asarray(v).astype(E4), np.float64)


def _features(probs, image):
    ys, xs = np.meshgrid(np.arange(H, dtype=np.float64),
                         np.arange(W, dtype=np.float64), indexing="ij")
    y = ys.ravel()[PERM]
    x = xs.ravel()[PERM]
    col = image[0].astype(np.float64).reshape(3, N)[:, PERM]
    a = probs[0, 0].astype(np.float64).reshape(N)[PERM]
    b = 1.0 - a

    rC1 = np.sqrt(C1)
    yt, xt, gt = rC1 * y, rC1 * x, (rC1 * LAM) * col
    base = yt * yt + xt * xt + (gt * gt).sum(axis=0)
    B1 = _f8(base); B2 = _f8(base - B1); B3 = _f8(base - B1 - B2)
    one = np.ones(N)
    U, V = [], []
    for t in (B1, B2, B3):
        U.append(t); V.append(one)
    for t in (B1, B2, B3):
        U.append(one); V.append(t)

    def cross(w):
        h = _f8(w); r = w - h; m = _f8(r); l = _f8(r - m)
        for ui, vj in [(h, h), (h, m), (m, h), (h, l), (l, h), (m, m)]:
            U.append(_f8(-2.0 * ui)); V.append(vj)

    cross(yt); cross(xt)
    for ch in range(3):
        cross(gt[ch])
    U = np.stack(U).astype(E4)      # [36, N]
    V = np.stack(V).astype(E4)

    ah = _f8(a); al = _f8(a - ah); bh = _f8(b); bl = _f8(b - bh)
    stat = np.stack([ah, al, bh, bl], axis=1).astype(E4)   # [N, 4]
    return U, V, stat, a, b, y, x, col


def _host_diag(y, x, col, a, b):
    """K_ii diagonal plus the 18 in-chunk 512x512 upper triangles (fp64)."""
    tot = float((a * b).sum())
    iu = np.triu_indices(JC, k=1)
    for m in range(N_CHUNKS):
        sl = slice(m * JC, (m + 1) * JC)
        yy, xx, aa, bb = y[sl], x[sl], a[sl], b[sl]
        cc = col[:, sl]
        dxy = (yy[:, None] - yy[None, :]) ** 2 + (xx[:, None] - xx[None, :]) ** 2
        drgb = ((cc[:, :, None] - cc[:, None, :]) ** 2).sum(axis=0)
        K = np.exp(-C1 * dxy - C2 * drgb)
        w = aa[:, None] * bb[None, :] + bb[:, None] * aa[None, :]
        tot += float((w[iu] * K[iu]).sum())
    return tot


def kernel(probs: np.ndarray, image: np.ndarray) -> np.ndarray:
    probs = np.asarray(probs)
    image = np.asarray(image)
    assert probs.shape == (1, 2, H, W) and image.shape == (1, 3, H, W)

    if "nc" not in _CACHE:
        _CACHE["nc"] = _build_program()
    nc = _CACHE["nc"]

    U, V, stat, a, b, y, x, col = _features(probs, image)

    vf = np.zeros((KPART, N), dtype=E4)
    for o, m in enumerate(COL_ORDER):
        vf[:RANK, o * JC:(o + 1) * JC] = V[:, m * JC:(m + 1) * JC]

    in_maps = []
    for c in range(N_CORES):
        uf = np.zeros((KPART, NSLOTS * IT), dtype=E4)
        st = np.zeros((128, NSLOTS, 16), dtype=E4)
        g = 0
        for o, m in enumerate(COL_ORDER):
            for s in range(S_O[o]):
                t = ASSIGN[m][c, s]
                if t >= 0:
                    iw = slice(t * IT, (t + 1) * IT)
                    uf[:RANK, g * IT:(g + 1) * IT] = U[:, iw]
                    st[:, g, 0:4] = stat[iw, :]
                g += 1
        in_maps.append({"uf": uf, "vf": vf, "st": st})
    _CACHE["in_maps"] = in_maps

    res = run_bass_kernel_spmd(nc, in_maps, list(range(N_CORES)))

    tri = np.float64(0.0)
    for c in range(N_CORES):
        stage = res.results[c]["stage"].astype(np.float64)   # [4, 17*512]
        for o, m in enumerate(COL_ORDER):
            jw = slice(m * JC, (m + 1) * JC)
            r = stage[:, o * JC:(o + 1) * JC]
            tri += ((r[0] + r[1]) * b[jw]).sum() + ((r[2] + r[3]) * a[jw]).sum()

    tri /= KSCALE
    tri += _host_diag(y, x, col, a, b)
    loss = 2.0 * tri / N
    return np.float32(loss)


# revision 17
# speedup vs baseline: 1.0708x; 1.0066x over previous
"""Dense CRF pairwise loss on 8 Trainium2 NeuronCores.

loss = (2/N) * [ sum_{i<j} (a_i b_j + a_j b_i) K_ij + sum_i a_i b_i ],
a = probs[:,0], b = 1-a, K_ij = exp(-c1*d_xy - c2*d_rgb), K_ii = 1.

Pixels are permuted into 8x16 patches (i-tiles of 128) grouped into 16x32
chunks (j-columns of 512).  The device computes the strictly-off-diagonal
upper-triangle blocks (i-tile t vs chunk m, t < 4m) whose patch boxes are
within RCUT=33 px (the Gaussian tail beyond is negligible): 55 slots/core.
The 18 block-diagonal 512x512 triangles and the K_ii=1 diagonal are done
on host in numpy (~3% of pairs) -- no masks needed on device.

The exponent x = c1*d_xy + c2*d_rgb is ONE fp8e4m3 matmul per block:
features are pre-scaled by sqrt(c1) and hi/mid/lo-split so every value is
e4m3-exact; rank 36, zero-padded to a 96-partition contraction because the
PE runs 2 cols/cycle only when the contraction dim is >= ~96 (measured:
K=96 -> 242 ns per 512-col matmul, K<=88 -> 443 ns).  ScalarE applies
exp(-x + ln 128) writing K~ = 128*K in fp8 (the 2^7 scale preserves
small-K mass against the e4m3 subnormal floor).

Per column, per-slot stats [ah al bh bl] (fp8, 16B-padded) reduce K~ over
i with fp8 DoubleRow r-matmuls, two slots per matmul ([128,2,4]^T @
[128,2,512] -> [4,512], 248 ns = 2 fp8 cols/cycle); odd tails use a plain
fp8 matmul.  DVE copies each column's [4,512] PSUM accumulator to a stage
buffer; the stage is DMAd out and dotted with [b,b,a,a] on host.

SPMD: all cores run the identical 55-slot schedule; slot -> i-tile is
per-core data.  Padding slots have zero features and zero stats
(exp -> 128, stats 0 -> no contribution).
"""

import numpy as np
import ml_dtypes

import concourse.bass as bass
import concourse.tile as tile
from concourse import bacc, mybir
from concourse.bass_utils import run_bass_kernel_spmd

E4 = ml_dtypes.float8_e4m3

H = W = 96
N = H * W                       # 9216
N_CORES = 8
JC = 512                        # column width (one 16x32 chunk)
N_CHUNKS = 18
IT = 128                        # i-tile (8x16 patch)
KPART = 96                      # matmul contraction partitions (36 real)
RANK = 36
RCUT = 33.0                     # patch-box cull radius (px)
KSCALE = 128.0                  # K stored as 128*K in fp8
LN_KSCALE = float(np.log(KSCALE))

SIGMA_XY = 15.0
SIGMA_RGB = 0.125
C1 = 1.0 / (2.0 * SIGMA_XY * SIGMA_XY)
C2 = 1.0 / (2.0 * SIGMA_RGB * SIGMA_RGB)
LAM = np.sqrt(C2 / C1)          # 120

_CACHE = {}


# ---------------- geometry: patches, chunks, cull, schedule ----------------

def _geometry():
    boxes = []          # per patch (y0,y1,x0,x1) inclusive
    perm = []           # new pixel index -> original row-major index
    for cy in range(6):
        for cx in range(3):
            for py in range(2):
                for px in range(2):
                    y0, x0 = cy * 16 + py * 8, cx * 32 + px * 16
                    boxes.append((y0, y0 + 7, x0, x0 + 15))
                    for yy in range(y0, y0 + 8):
                        for xx in range(x0, x0 + 16):
                            perm.append(yy * 96 + xx)
    perm = np.array(perm)
    cbox = [(min(boxes[4 * m + q][0] for q in range(4)),
             max(boxes[4 * m + q][1] for q in range(4)),
             min(boxes[4 * m + q][2] for q in range(4)),
             max(boxes[4 * m + q][3] for q in range(4))) for m in range(N_CHUNKS)]

    def bdist(b1, b2):
        dy = max(0, b1[0] - b2[1], b2[0] - b1[1])
        dx = max(0, b1[2] - b2[3], b2[2] - b1[3])
        return float(np.hypot(dy, dx))

    # strictly-off-diagonal kept blocks only (t < 4m); diagonal on host
    kept = {m: [t for t in range(4 * m)
                if bdist(boxes[t], cbox[m]) <= RCUT] for m in range(N_CHUNKS)}
    cols = [m for m in range(N_CHUNKS) if kept[m]]
    s_m = {m: -(-len(kept[m]) // N_CORES) for m in cols}
    # big columns first (DMA streaming + short tail), small columns last
    col_order = sorted(cols, key=lambda m: -s_m[m])
    s_o = [s_m[m] for m in col_order]

    assign = {}
    for m in cols:
        A = -np.ones((N_CORES, s_m[m]), int)
        for k, t in enumerate(kept[m]):
            A[k % N_CORES, k // N_CORES] = t
        assign[m] = A
    return perm, col_order, s_o, assign


PERM, COL_ORDER, S_O, ASSIGN = _geometry()
N_COLS = len(COL_ORDER)         # 17
NSLOTS = sum(S_O)               # 55


# ---------------- device program ----------------

def _build_program():
    nc = bacc.Bacc("TRN2", target_bir_lowering=False, debug=False)
    f32 = mybir.dt.float32
    fp8 = mybir.dt.float8e4
    DR = mybir.MatmulPerfMode.DoubleRow

    uf_d = nc.dram_tensor("uf", [KPART, NSLOTS * IT], fp8, kind="ExternalInput")
    vf_d = nc.dram_tensor("vf", [KPART, N], fp8, kind="ExternalInput")
    st_d = nc.dram_tensor("st", [128, NSLOTS, 16], fp8, kind="ExternalInput")
    stage_d = nc.dram_tensor("stage", [4, N_COLS * JC], f32, kind="ExternalOutput")

    base_o = np.concatenate([[0], np.cumsum(S_O)]).astype(int)

    with tile.TileContext(nc) as tc:
        with (
            tc.tile_pool(name="const", bufs=1) as cpool,
            tc.tile_pool(name="kgrp", bufs=2) as kpool,
            tc.tile_pool(name="pse", bufs=3, space="PSUM") as pe_pool,
            tc.tile_pool(name="psr", bufs=2, space="PSUM") as pr_pool,
        ):
            uf_t = cpool.tile([KPART, NSLOTS * IT], fp8)
            vf_t = cpool.tile([KPART, N], fp8)
            st_t = cpool.tile([128, NSLOTS, 16], fp8)
            stage_t = cpool.tile([4, N_COLS * JC], f32)
            warm_t = cpool.tile([KPART, JC], fp8)
            bias_t = cpool.tile([128, 1], f32)

            # --- input DMAs (sync/gpsimd/scalar queues), first-needed first
            nc.gpsimd.dma_start(uf_t[:, 0:3 * IT], uf_d.ap()[:, 0:3 * IT])
            nc.gpsimd.dma_start(uf_t[:, 3 * IT:10 * IT],
                                uf_d.ap()[:, 3 * IT:10 * IT])
            nc.gpsimd.dma_start(uf_t[:, 10 * IT:28 * IT],
                                uf_d.ap()[:, 10 * IT:28 * IT])
            nc.gpsimd.dma_start(uf_t[:, 28 * IT:], uf_d.ap()[:, 28 * IT:])
            nc.sync.dma_start(vf_t[:, 0:JC], vf_d.ap()[:, 0:JC])
            nc.sync.dma_start(vf_t[:, JC:4 * JC], vf_d.ap()[:, JC:4 * JC])
            nc.sync.dma_start(st_t[:, 0:8, :], st_d.ap()[:, 0:8, :])
            nc.sync.dma_start(st_t[:, 8:, :], st_d.ap()[:, 8:, :])
            nc.scalar.dma_start(vf_t[:, 4 * JC:], vf_d.ap()[:, 4 * JC:])
            nc.vector.memset(warm_t[:], 0.0)
            nc.vector.memset(bias_t[:], LN_KSCALE)

            # --- PE warm-up while DMAs land ---
            warm_ps = pe_pool.tile([128, 2, JC], f32, tag="pse", name="warm_ps")
            for _ in range(2):
                nc.tensor.matmul(warm_ps[:, 0, :], warm_t[:, 0:IT], warm_t[:],
                                 start=True, stop=True)

            # --- main pipeline: columns in order, r-matmuls one column behind
            pending = []
            done = [0]

            def flush(o, s, kbuf, psr_t):
                npair = s // 2
                for p in range(npair):
                    g = base_o[o] + 2 * p
                    nc.tensor.matmul(
                        psr_t[:], st_t[:, g:g + 2, 0:4], kbuf[:, 2 * p:2 * p + 2, :],
                        start=(p == 0), stop=(p == npair - 1 and s % 2 == 0),
                        perf_mode=DR)
                if s % 2:
                    g = base_o[o] + s - 1
                    nc.tensor.matmul(
                        psr_t[:], st_t[:, g, 0:4], kbuf[:, s - 1, :],
                        start=(s == 1), stop=True)
                nc.vector.tensor_copy(stage_t[:, o * JC:(o + 1) * JC], psr_t[:])
                nc.sync.dma_start(stage_d.ap()[:, o * JC:(o + 1) * JC],
                                  stage_t[:, o * JC:(o + 1) * JC])
                done[0] += 1

            for o in range(N_COLS):
                s = S_O[o]
                kbuf = kpool.tile([128, 5, JC], fp8, tag="kg", name=f"kb{o}")
                psr_t = pr_pool.tile([4, JC], f32, tag="psr", name=f"pr{o}")
                for g0 in range(0, s, 2):
                    ln = min(2, s - g0)
                    ps = pe_pool.tile([128, 2, JC], f32, tag="pse")
                    for u in range(ln):
                        gslot = base_o[o] + g0 + u
                        nc.tensor.matmul(
                            ps[:, u, :],
                            uf_t[:, gslot * IT:(gslot + 1) * IT],
                            vf_t[:, o * JC:(o + 1) * JC],
                            start=True, stop=True)
                    nc.scalar.activation(
                        kbuf[:, g0:g0 + ln, :], ps[:, 0:ln, :],
                        mybir.ActivationFunctionType.Exp,
                        scale=-1.0, bias=bias_t[:, 0:1])
                pending.append((o, s, kbuf, psr_t))
                if len(pending) > 1:
                    flush(*pending.pop(0))
            while pending:
                flush(*pending.pop(0))

    nc.compile()
    return nc


# ---------------- host-side features ----------------

def _f8(v):
    return np.asarray(np.asarray(v).astype(E4), np.float64)


def _features(probs, image):
    ys, xs = np.meshgrid(np.arange(H, dtype=np.float64),
                         np.arange(W, dtype=np.float64), indexing="ij")
    y = ys.ravel()[PERM]
    x = xs.ravel()[PERM]
    col = image[0].astype(np.float64).reshape(3, N)[:, PERM]
    a = probs[0, 0].astype(np.float64).reshape(N)[PERM]
    b = 1.0 - a

    rC1 = np.sqrt(C1)
    yt, xt, gt = rC1 * y, rC1 * x, (rC1 * LAM) * col
    base = yt * yt + xt * xt + (gt * gt).sum(axis=0)
    B1 = _f8(base); B2 = _f8(base - B1); B3 = _f8(base - B1 - B2)
    one = np.ones(N)
    U, V = [], []
    for t in (B1, B2, B3):
        U.append(t); V.append(one)
    for t in (B1, B2, B3):
        U.append(one); V.append(t)

    def cross(w):
        h = _f8(w); r = w - h; m = _f8(r); l = _f8(r - m)
        for ui, vj in [(h, h), (h, m), (m, h), (h, l), (l, h), (m, m)]:
            U.append(_f8(-2.0 * ui)); V.append(vj)

    cross(yt); cross(xt)
    for ch in range(3):
        cross(gt[ch])
    U = np.stack(U).astype(E4)      # [36, N]
    V = np.stack(V).astype(E4)

    ah = _f8(a); al = _f8(a - ah); bh = _f8(b); bl = _f8(b - bh)
    stat = np.stack([ah, al, bh, bl], axis=1).astype(E4)   # [N, 4]
    return U, V, stat, a, b, y, x, col


def _host_diag(y, x, col, a, b):
    """K_ii diagonal plus the 18 in-chunk 512x512 upper triangles (fp64)."""
    tot = float((a * b).sum())
    iu = np.triu_indices(JC, k=1)
    for m in range(N_CHUNKS):
        sl = slice(m * JC, (m + 1) * JC)
        yy, xx, aa, bb = y[sl], x[sl], a[sl], b[sl]
        cc = col[:, sl]
        dxy = (yy[:, None] - yy[None, :]) ** 2 + (xx[:, None] - xx[None, :]) ** 2
        drgb = ((cc[:, :, None] - cc[:, None, :]) ** 2).sum(axis=0)
        K = np.exp(-C1 * dxy - C2 * drgb)
        w = aa[:, None] * bb[None, :] + bb[:, None] * aa[None, :]
        tot += float((w[iu] * K[iu]).sum())
    return tot


def kernel(probs: np.ndarray, image: np.ndarray) -> np.ndarray:
    probs = np.asarray(probs)
    image = np.asarray(image)
    assert probs.shape == (1, 2, H, W) and image.shape == (1, 3, H, W)

    if "nc" not in _CACHE:
        _CACHE["nc"] = _build_program()
    nc = _CACHE["nc"]

    U, V, stat, a, b, y, x, col = _features(probs, image)

    vf = np.zeros((KPART, N), dtype=E4)
    for o, m in enumerate(COL_ORDER):
        vf[:RANK, o * JC:(o + 1) * JC] = V[:, m * JC:(m + 1) * JC]

    in_maps = []
    for c in range(N_CORES):
        uf = np.zeros((KPART, NSLOTS * IT), dtype=E4)
        st = np.zeros((128, NSLOTS, 16), dtype=E4)
        g = 0
        for o, m in enumerate(COL_ORDER):
            for s in range(S_O[o]):
                t = ASSIGN[m][c, s]
                if t >= 0:
                    iw = slice(t * IT, (t + 1) * IT)
                    uf[:RANK, g * IT:(g + 1) * IT] = U[:, iw]
                    st[:, g, 0:4] = stat[iw, :]
                g += 1
        in_maps.append({"uf": uf, "vf": vf, "st": st})
    _CACHE["in_maps"] = in_maps

    res = run_bass_kernel_spmd(nc, in_maps, list(range(N_CORES)))

    tri = np.float64(0.0)
    for c in range(N_CORES):
        stage = res.results[c]["stage"].astype(np.float64)   # [4, 17*512]
        for o, m in enumerate(COL_ORDER):
            jw = slice(m * JC, (m + 1) * JC)
            r = stage[:, o * JC:(o + 1) * JC]
            tri += ((r[0] + r[1]) * b[jw]).sum() + ((r[2] + r[3]) * a[jw]).sum()

    tri /= KSCALE
    tri += _host_diag(y, x, col, a, b)
    loss = 2.0 * tri / N
    return np.float32(loss)
